# revision 38
# baseline (speedup 1.0000x reference)
"""LoRA cross-attention (self-attn) processor on 8 TRN2 NeuronCores.

Problem: B=4, S=2048, D=640, H=8 heads (hd=80), LoRA rank 4.
  q/k/v = x @ (W + up@down).T ; per-head attention; out = attn @ (Wo + o_up@o_down).T + bo

Sharding: batch*head parallel. Core c -> batch b=c//2, head-group g=c%2
(4 heads). Host folds the rank-4 LoRA updates into the weights (exact
algebra) and sums the two partial outputs per batch + bias at the end.

Schedule (all matmul operands bf16, PSUM fp32). The kernel is co-limited
by the PE (~370k matmul columns ~ 154us at 2.4GHz) and the Scalar
engine (128 exp instructions over 16.8M scores ~ 142us), under a chip
power throttle (~0.85 avg util). Design points:
  - q/k projection runs packed (M=128 per matmul, 5 output tiles of the
    640 q|k dims), then SBUF->SBUF DMA repacks rows into per-head-half
    [80, S] tiles (DMA has no partition-alignment restriction; engine
    APs would need 32-aligned bases, impossible at 80-row strides).
    Head 0's q stays in the packed tile and is read in place.
  - q-chunks of 1024: scores tiles [128k, 1024q] in 2-bank PSUM tiles,
    one ACT exp per tile (halves ACT per-instruction overhead vs 512).
    ACT runs exp only; copies go to DVE (or ACT when it is idle).
  - v stationary [128, 97] with a ones column at col 96: PV emits the
    softmax denominators at PSUM partition 96 (32-aligned, directly
    readable). Normalize = DVE copy + reciprocal_approx_fast [1,1024]
    + GPSIMD partition_broadcast + DVE multiplies; fp32 throughout.
  - attention output packed at 96-partition stride across 3 SBUF tiles
    (every head-segment boundary lands 32-aligned); output projection
    contracts 3x128 packed rows against host-zero-padded W_o (30720
    columns instead of 40960).
  - inputs stream on three DMA queues (SP: x halves c0-first, ACT
    HWDGE: w_qk, GPSIMD SWDGE: w_v/w_o) so the first projection chain
    starts ~2us in; upfront work is only head-0's needs + the v
    projection; all other projections, repacks and the first
    out-projection pass are fed into attention-loop insert slots so the
    PE never idles long enough for the HAM clock gate to drop.
  - out-projection accumulation is ordered ti=0,1,2 and paired so the
    steps depending on the last chunk's normalize land last.
  - output written bf16 (host accumulates the two partials in fp32).

PSUM = exactly 8 banks: tags sc(x2) + at(x2), all [128,1024] fp32.
Measured (NTFF, core 0): ~250us total / ~230us active vs 359us baseline.
"""
import numpy as np
import ml_dtypes

B, S, D, H, HD, R = 4, 2048, 640, 8, 80, 4
HPC = H // 2          # heads per core
GDIM = HPC * HD       # 320 head-dims per core
PAD = 96              # packed head stride (all segment bases 32-aligned)
NCORES = 8
NKT = S // 128        # 16 key tiles
NCT = D // 128        # 5 contraction tiles
VW = 97               # v stationary width: 80 dims + pad + ones col at 96
SM_SCALE = 1.0 / float(np.sqrt(HD))

_cache = {}


def _seg_legal(base, n):
    # Engine partition-range rule: n<=32 from any 32-aligned base,
    # n<=64 from {0,64}, larger only from 0.
    if n <= 32:
        return base % 32 == 0
    if n <= 64:
        return base in (0, 64)
    return base == 0


def _split_legal(pairs):
    """Split (off_a, off_b, n) ranges so every piece is engine-legal in
    both coordinates (all offsets here are multiples of 32, so one
    32-grid serves both)."""
    pieces = []
    for a, b, n in pairs:
        o = 0
        while o < n:
            m = n - o
            while m > 1 and not (_seg_legal(a + o, m) and _seg_legal(b + o, m)):
                m = 32 * ((m - 1) // 32) if m > 32 else m - 1
            pieces.append((a + o, b + o, m))
            o += m
    return pieces


def _head_segments(h):
    """Packed-96 attn layout: head h occupies packed rows [96h, 96h+80)
    across three 128-partition tiles -> (tile, tile_lo, at_lo, n)."""
    segs = []
    lo, hi = PAD * h, PAD * h + HD
    for t in range(3):
        s, e = max(lo, 128 * t), min(hi, 128 * (t + 1))
        if s < e:
            for tl, al, n in _split_legal([(s - 128 * t, s - lo, e - s)]):
                segs.append((t, tl, al, n))
    return segs


def _qk_dma_splits(ti):
    """Packed q|k projection tile ti covers global dims [128ti, 128ti+128):
    rows 80j..80j+80 belong to head-half j (0..3 q heads, 4..7 k heads).
    Returns (src_lo, hh, dst_lo, n) DMA copies."""
    out = []
    glo, ghi = 128 * ti, 128 * (ti + 1)
    j0, j1 = glo // HD, (ghi - 1) // HD
    for j in range(j0, j1 + 1):
        s, e = max(glo, HD * j), min(ghi, HD * (j + 1))
        out.append((s - glo, j, s - HD * j, e - s))
    return out


def _body(tc, xT, w_qk, w_v, w_o, outT, dbg=None):
    import concourse.mybir as mybir

    nc = tc.nc
    bf = mybir.dt.bfloat16
    f32 = mybir.dt.float32
    Exp = mybir.ActivationFunctionType.Exp

    with tc.tile_pool(name="weights", bufs=1) as wpool, \
         tc.tile_pool(name="persist", bufs=1) as pers, \
         tc.tile_pool(name="psum", bufs=2, space="PSUM") as ps, \
         tc.tile_pool(name="work", bufs=1) as work:
        # ---- input DMAs on two HWDGE queues: weights on the (idle) ACT
        # queue, x on the SP queue, c2=0 halves first so the upfront
        # projections can start ~6us in ----
        xT_t, wqk_t = [], []
        for i in range(NCT):
            t = wpool.tile([128, S], bf, name=f"xT{i}", tag=f"xT{i}")
            # x tiles 3,4 ride the ACT queue: the SP queue alone gates the
            # first projection chain on tile 4's arrival.
            eng = nc.sync if i < 3 else nc.scalar
            eng.dma_start(out=t[:, 0:1024],
                          in_=xT[128 * i:128 * (i + 1), 0:1024])
            xT_t.append(t)
            t = wpool.tile([128, 2 * GDIM], bf, name=f"wqk{i}", tag=f"wqk{i}")
            nc.scalar.dma_start(out=t, in_=w_qk[128 * i:128 * (i + 1), :])
            wqk_t.append(t)
        for i in range(NCT):
            nc.sync.dma_start(out=xT_t[i][:, 1024:2048],
                              in_=xT[128 * i:128 * (i + 1), 1024:2048])
        wv_t = []
        for i in range(NCT):
            t = wpool.tile([128, GDIM], bf, name=f"wv{i}", tag=f"wv{i}")
            nc.gpsimd.dma_start(out=t, in_=w_v[128 * i:128 * (i + 1), :])
            wv_t.append(t)
        wo_t = []
        for i in range(3):
            t = wpool.tile([128, D], bf, name=f"wo{i}", tag=f"wo{i}")
            nc.gpsimd.dma_start(out=t, in_=w_o[128 * i:128 * (i + 1), :])
            wo_t.append(t)

        # ---- persistent SBUF ----
        qkT = [pers.tile([HD, S], bf, name=f"qkT{i}", tag=f"qkT{i}")
               for i in range(2 * HPC)]
        vaug = [pers.tile([128, VW * NKT], bf, name=f"va{h}", tag=f"va{h}")
                for h in range(HPC)]
        for h in range(HPC):
            nc.vector.memset(vaug[h], 0.0)
            nc.vector.memset(vaug[h][:, VW - 1:VW * NKT:VW], 1.0)
        # packed-96 normalized attention, 3 tiles of [128, S]. Zeroed so the
        # never-written pad rows can't feed NaN into the out-projection
        # (its weights there are zero, but 0*NaN = NaN on the PE).
        attn_p = [pers.tile([128, S], bf, name=f"ap{t}", tag=f"ap{t}")
                  for t in range(3)]
        for t in range(3):
            nc.vector.memset(attn_p[t], 0.0)
        # head 0's q chunks stay in these tiles (rows 0:80) — no repack.
        qk0c = [pers.tile([128, 1024], bf, name=f"qk0c{c}", tag=f"qk0c{c}")
                for c in range(2)]

        def emit_qk_proj(ti, c2, on_act=False):
            # packed q|k projection: output dims [128ti, 128ti+128), one
            # 1024-wide query chunk; copy to SBUF, DMA repack per head.
            # on_act: use the Scalar engine for the copy (idle pre-attention).
            sc = ps.tile([128, 1024], f32, name="qkps", tag="sc")
            for half in range(2):
                cs = slice(1024 * c2 + 512 * half, 1024 * c2 + 512 * (half + 1))
                for k in range(NCT):
                    nc.tensor.matmul(
                        sc[:, 512 * half:512 * (half + 1)],
                        wqk_t[k][:, 128 * ti:128 * (ti + 1)], xT_t[k][:, cs],
                        start=(k == 0), stop=(k == NCT - 1))
            if ti == 0:
                qkp = qk0c[c2]   # head 0 reads rows 0:80 in place
            else:
                qkp = work.tile([128, 1024], bf, name="qkp", tag="qkp", bufs=2)
            if on_act:
                nc.scalar.copy(out=qkp, in_=sc)
            else:
                nc.vector.tensor_copy(out=qkp, in_=sc)
            for slo, hh, dlo, n in _qk_dma_splits(ti):
                if ti == 0 and hh == 0:
                    continue
                nc.sync.dma_start(
                    out=qkT[hh][dlo:dlo + n, 1024 * c2:1024 * (c2 + 1)],
                    in_=qkp[slo:slo + n, :])

        def emit_v_proj(kt):
            # v projection for seq tile kt -> vaug[h] stationaries; copies
            # split ACT/DVE so neither engine paces the PE here.
            sc = ps.tile([128, 1024], f32, name="vps", tag="sc")
            for k in range(NCT):
                nc.tensor.matmul(sc[:, 0:GDIM], xT_t[k][:, 128 * kt:128 * (kt + 1)],
                                 wv_t[k], start=(k == 0), stop=(k == NCT - 1))
            for h in range(HPC):
                eng = nc.scalar if h < 2 else nc.vector
                if eng is nc.scalar:
                    nc.scalar.copy(out=vaug[h][:, VW * kt:VW * kt + HD],
                                   in_=sc[:, HD * h:HD * (h + 1)])
                else:
                    nc.vector.tensor_copy(out=vaug[h][:, VW * kt:VW * kt + HD],
                                          in_=sc[:, HD * h:HD * (h + 1)])

        def emit_out_proj(dt, c2, pair=None):
            # Accumulation order ti=0,1,2: only ti=2 (attn tiles holding head
            # 3) depends on the final chunk's normalize — emitting the ti<2
            # steps of two tiles first hides that latency (see tail callers).
            def chain(po, dt_, tis):
                for ti in tis:
                    nc.tensor.matmul(
                        po[:, :512], wo_t[ti][:, 128 * dt_:128 * (dt_ + 1)],
                        attn_p[ti][:, 1024 * c2:1024 * c2 + 512],
                        start=(ti == 0), stop=(ti == 2))
                    nc.tensor.matmul(
                        po[:, 512:], wo_t[ti][:, 128 * dt_:128 * (dt_ + 1)],
                        attn_p[ti][:, 1024 * c2 + 512:1024 * (c2 + 1)],
                        start=(ti == 0), stop=(ti == 2))

            def finish(po, dt_):
                chain(po, dt_, (2,))
                ob = work.tile([128, 1024], bf, name="ob", tag="ob", bufs=3)
                nc.vector.tensor_copy(out=ob, in_=po)
                nc.sync.dma_start(
                    out=outT[128 * dt_:128 * (dt_ + 1),
                             1024 * c2:1024 * (c2 + 1)],
                    in_=ob)

            po = ps.tile([128, 1024], f32, name="po", tag="sc")
            chain(po, dt, (0, 1))
            if pair is None:
                finish(po, dt)
            else:
                po2 = ps.tile([128, 1024], f32, name="po2", tag="sc")
                chain(po2, pair, (0, 1))
                finish(po, dt)
                finish(po2, pair)

        # upfront: what scores (h0, c2=0, kt<8) need — q0 cols 0:1024
        # (tile 0), k0 cols 0:1024 (tiles 2,3) — plus the whole v
        # projection (PE runs it back-to-back; inside the attention loop
        # it would serialize against the exp stream).
        for ti, c2 in ((0, 0), (2, 0), (3, 0)):
            emit_qk_proj(ti, c2, on_act=True)
        for kt in range(NKT):
            emit_v_proj(kt)

        # insertable work, popped at fixed kt slots of the attention loop.
        def qk(ti, c2):
            return lambda: emit_qk_proj(ti, c2)

        # Insert placement: chunks 0-1 are PE-bound anyway (required
        # projections), so extra work there is free; chunks 2-7 run with
        # ACT ~98% saturated and any inserted PE work would delay the
        # in-order scores stream and stall the exp pipeline - keep them
        # clean except the late out-projection tiles.
        queue = [
            # chunk 0 (h0,c0): k0 cols 1024:2048 (needed from kt=8),
            # q1 rows for (h1,c0)
            [qk(2, 1), qk(3, 1), qk(1, 0)],
            # chunk 1 (h1,c0): q c2=1 tiles (chunk 4), k heads 2,3 (chunk 2)
            [qk(0, 1), qk(1, 1), qk(4, 0), qk(4, 1)],
            [], [], [],          # (h2,c0), (h3,c0), (h0,c1)
            [lambda: emit_out_proj(0, 0), lambda: emit_out_proj(1, 0)],
            [lambda: emit_out_proj(2, 0)],
            [],
        ]

        ci = 0
        for c2 in range(2):
            for h in range(HPC):
                inserts = queue[ci] if ci < len(queue) else []
                ci += 1
                slots = {1: 0, 3: 1, 7: 2, 11: 3, 14: 4}
                at_ = ps.tile([128, 1024], f32, name="at", tag="at")
                pbs = []
                for kt in range(NKT):
                    sc = ps.tile([128, 1024], f32, name="sc", tag="sc")
                    for half in range(2):
                        if h == 0:
                            rhs = qk0c[c2][0:HD, 512 * half:512 * (half + 1)]
                        else:
                            rhs = qkT[h][:, 1024 * c2 + 512 * half:
                                         1024 * c2 + 512 * (half + 1)]
                        nc.tensor.matmul(
                            sc[:, 512 * half:512 * (half + 1)],
                            qkT[HPC + h][:, 128 * kt:128 * (kt + 1)],
                            rhs, start=True, stop=True)
                    pb = work.tile([128, 1024], bf, name="pb", tag=f"pb{kt % 6}",
                                   bufs=1)
                    nc.scalar.activation(out=pb, in_=sc, func=Exp, scale=SM_SCALE)
                    pbs.append(pb)
                    if kt > 0:
                        _emit_pv(nc, at_, vaug[h], pbs[kt - 1], kt - 1)
                    if kt in slots and slots[kt] < len(inserts):
                        fn = inserts[slots[kt]]
                        if fn is not None:
                            fn()
                _emit_pv(nc, at_, vaug[h], pbs[NKT - 1], NKT - 1)
                # normalize: denominators sit at PSUM partition 96. For the
                # last chunk split into 512-halves to halve the tail latency.
                nhalves = 2 if (c2 == 1 and h == HPC - 1) else 1
                w_ = 1024 // nhalves
                for hv in range(nhalves):
                    cs = slice(w_ * hv, w_ * (hv + 1))
                    rdin = work.tile([1, w_], f32, name="rdin", tag="rdin", bufs=2)
                    nc.vector.tensor_copy(out=rdin, in_=at_[96:97, cs])
                    rdr = work.tile([1, w_], f32, name="rdr", tag="rdr", bufs=2)
                    with nc.allow_low_precision(reason="softmax recip, 51 ULP"):
                        nc.vector.reciprocal_approx_fast(out=rdr, in_=rdin)
                    rb = work.tile([HD, w_], f32, name="rb", tag="rb", bufs=2)
                    nc.gpsimd.partition_broadcast(rb, rdr)
                    for t, tlo, alo, n in _head_segments(h):
                        nc.vector.tensor_mul(
                            out=attn_p[t][tlo:tlo + n,
                                          1024 * c2 + w_ * hv:
                                          1024 * c2 + w_ * (hv + 1)],
                            in0=at_[alo:alo + n, cs], in1=rb[alo:alo + n, :])
                if dbg is not None and h == 0 and c2 == 0:
                    atc = work.tile([128, 1024], f32, name="atc", tag="atc")
                    nc.vector.tensor_copy(out=atc, in_=at_)
                    nc.sync.dma_start(out=dbg["at0"], in_=atc)
                    nc.sync.dma_start(out=dbg["rdr0"], in_=rdr)
                    nc.sync.dma_start(out=dbg["rb0"], in_=rb)

        # tail: the last two c2=0 output tiles cover the final chunk's
        # normalize latency, then the c2=1 output projection with paired
        # chains so the head-3-dependent steps land last.
        for dt in (3, 4):
            emit_out_proj(dt, 0)
        emit_out_proj(0, 1, pair=1)
        emit_out_proj(2, 1, pair=3)
        emit_out_proj(4, 1)


def _emit_pv(nc, at_, vaug_h, pb, kt):
    for half in range(2):
        nc.tensor.matmul(
            at_[0:VW, 512 * half:512 * (half + 1)],
            vaug_h[:, VW * kt:VW * (kt + 1)],
            pb[:, 512 * half:512 * (half + 1)],
            start=(kt == 0), stop=(kt == NKT - 1))


def build_nc(loop=1, debug=False):
    import concourse.mybir as mybir
    import concourse.tile as tile
    from concourse import bacc

    bf = mybir.dt.bfloat16
    f32 = mybir.dt.float32
    nc = bacc.Bacc("TRN2", target_bir_lowering=False, debug=False,
                   num_devices=NCORES)
    xT = nc.dram_tensor("xT", [D, S], bf, kind="ExternalInput").ap()
    w_qk = nc.dram_tensor("w_qk", [D, 2 * GDIM], bf, kind="ExternalInput").ap()
    w_v = nc.dram_tensor("w_v", [D, GDIM], bf, kind="ExternalInput").ap()
    w_o = nc.dram_tensor("w_o", [3 * 128, D], bf, kind="ExternalInput").ap()
    outT = nc.dram_tensor("outT", [D, S], bf, kind="ExternalOutput").ap()
    dbg = None
    if debug:
        dbg = {
            "at0": nc.dram_tensor("at0", [128, 1024], f32, kind="ExternalOutput").ap(),
            "rdr0": nc.dram_tensor("rdr0", [1, 1024], f32, kind="ExternalOutput").ap(),
            "rb0": nc.dram_tensor("rb0", [HD, 1024], f32, kind="ExternalOutput").ap(),
        }
    with tile.TileContext(nc) as tc:
        if loop == 1:
            _body(tc, xT, w_qk, w_v, w_o, outT, dbg)
        else:
            with tc.For_i(0, loop, 1):
                _body(tc, xT, w_qk, w_v, w_o, outT)
    nc.compile()
    return nc


def make_in_maps(inputs):
    """Host-side shard + layout prep. inputs: full-size fp32 arrays."""
    f = {k: np.asarray(v, dtype=np.float64) for k, v in inputs.items()}
    w_eff = {}
    for nm in ("q", "k", "v", "o"):
        w_eff[nm] = (f[f"w{nm}"] + f[f"{nm}_up"] @ f[f"{nm}_down"])
    bfd = ml_dtypes.bfloat16
    x = f["hidden_states"]  # [B, S, D]
    in_maps = []
    for c in range(NCORES):
        b, g = divmod(c, 2)
        rows = slice(GDIM * g, GDIM * (g + 1))
        xT_ = np.ascontiguousarray(x[b].T).astype(bfd)
        wq = w_eff["q"][rows, :].T  # [640, 320]
        wk = w_eff["k"][rows, :].T
        w_qk = np.ascontiguousarray(np.concatenate([wq, wk], axis=1)).astype(bfd)
        w_v = np.ascontiguousarray(w_eff["v"][rows, :].T).astype(bfd)
        # packed-96 w_o: rows 96h..96h+80 = head h's 80 contraction rows,
        # pad rows zero so they contribute nothing.
        wo_pack = np.zeros((3 * 128, D), np.float64)
        for h in range(HPC):
            wo_pack[PAD * h:PAD * h + HD, :] = \
                w_eff["o"][:, GDIM * g + HD * h:GDIM * g + HD * (h + 1)].T
        in_maps.append({"xT": xT_, "w_qk": w_qk, "w_v": w_v,
                        "w_o": wo_pack.astype(bfd)})
    return in_maps


def assemble_out(results, bo):
    out = np.empty((B, S, D), np.float32)
    for b in range(B):
        pt = (results[2 * b]["outT"].astype(np.float32)
              + results[2 * b + 1]["outT"].astype(np.float32))  # [640, 2048]
        out[b] = pt.T + bo[None, :].astype(np.float32)
    return out


def kernel(**inputs):
    from concourse.bass_utils import run_bass_kernel_spmd

    if "nc" not in _cache:
        _cache["nc"] = build_nc()
    nc = _cache["nc"]
    in_maps = make_in_maps(inputs)
    res = run_bass_kernel_spmd(nc, in_maps, list(range(NCORES)))
    return assemble_out(res.results, np.asarray(inputs["bo"], np.float32))


# revision 39
# speedup vs baseline: 1.0080x; 1.0080x over previous
"""LoRA cross-attention (self-attn) processor on 8 TRN2 NeuronCores.

Problem: B=4, S=2048, D=640, H=8 heads (hd=80), LoRA rank 4.
  q/k/v = x @ (W + up@down).T ; per-head attention; out = attn @ (Wo + o_up@o_down).T + bo

Sharding: batch*head parallel. Core c -> batch b=c//2, head-group g=c%2
(4 heads). Host folds the rank-4 LoRA updates into the weights (exact
algebra) and sums the two partial outputs per batch + bias at the end.

Schedule (all matmul operands bf16, PSUM fp32). The kernel is co-limited
by the PE (~370k matmul columns ~ 154us at 2.4GHz) and the Scalar
engine (128 exp instructions over 16.8M scores ~ 142us), under a chip
power throttle (~0.85 avg util). Design points:
  - q/k projection runs packed (M=128 per matmul, 5 output tiles of the
    640 q|k dims), then SBUF->SBUF DMA repacks rows into per-head-half
    [80, S] tiles (DMA has no partition-alignment restriction; engine
    APs would need 32-aligned bases, impossible at 80-row strides).
    Head 0's q stays in the packed tile and is read in place.
  - q-chunks of 1024: scores tiles [128k, 1024q] in 2-bank PSUM tiles,
    one ACT exp per tile (halves ACT per-instruction overhead vs 512).
    ACT runs exp only; copies go to DVE (or ACT when it is idle).
  - v stationary [128, 97] with a ones column at col 96: PV emits the
    softmax denominators at PSUM partition 96 (32-aligned, directly
    readable). Normalize = DVE copy + reciprocal_approx_fast [1,1024]
    + GPSIMD partition_broadcast + DVE multiplies; fp32 throughout.
  - attention output packed at 96-partition stride across 3 SBUF tiles
    (every head-segment boundary lands 32-aligned); output projection
    contracts 3x128 packed rows against host-zero-padded W_o (30720
    columns instead of 40960).
  - inputs stream on three DMA queues (SP: x halves c0-first, ACT
    HWDGE: w_qk, GPSIMD SWDGE: w_v/w_o) so the first projection chain
    starts ~2us in; upfront work is only head-0's needs + the v
    projection; all other projections, repacks and the first
    out-projection pass are fed into attention-loop insert slots so the
    PE never idles long enough for the HAM clock gate to drop.
  - out-projection accumulation is ordered ti=0,1,2 and paired so the
    steps depending on the last chunk's normalize land last.
  - output written bf16 (host accumulates the two partials in fp32).

PSUM = exactly 8 banks: tags sc(x2) + at(x2), all [128,1024] fp32.
Measured (NTFF, core 0): ~250us total / ~230us active vs 359us baseline.
"""
import numpy as np
import ml_dtypes

B, S, D, H, HD, R = 4, 2048, 640, 8, 80, 4
HPC = H // 2          # heads per core
GDIM = HPC * HD       # 320 head-dims per core
PAD = 96              # packed head stride (all segment bases 32-aligned)
NCORES = 8
NKT = S // 128        # 16 key tiles
NCT = D // 128        # 5 contraction tiles
VW = 97               # v stationary width: 80 dims + pad + ones col at 96
SM_SCALE = 1.0 / float(np.sqrt(HD))

_cache = {}


def _seg_legal(base, n):
    # Engine partition-range rule: n<=32 from any 32-aligned base,
    # n<=64 from {0,64}, larger only from 0.
    if n <= 32:
        return base % 32 == 0
    if n <= 64:
        return base in (0, 64)
    return base == 0


def _split_legal(pairs):
    """Split (off_a, off_b, n) ranges so every piece is engine-legal in
    both coordinates (all offsets here are multiples of 32, so one
    32-grid serves both)."""
    pieces = []
    for a, b, n in pairs:
        o = 0
        while o < n:
            m = n - o
            while m > 1 and not (_seg_legal(a + o, m) and _seg_legal(b + o, m)):
                m = 32 * ((m - 1) // 32) if m > 32 else m - 1
            pieces.append((a + o, b + o, m))
            o += m
    return pieces


def _head_segments(h):
    """Packed-96 attn layout: head h occupies packed rows [96h, 96h+80)
    across three 128-partition tiles -> (tile, tile_lo, at_lo, n)."""
    segs = []
    lo, hi = PAD * h, PAD * h + HD
    for t in range(3):
        s, e = max(lo, 128 * t), min(hi, 128 * (t + 1))
        if s < e:
            for tl, al, n in _split_legal([(s - 128 * t, s - lo, e - s)]):
                segs.append((t, tl, al, n))
    return segs


def _qk_dma_splits(ti):
    """Packed q|k projection tile ti covers global dims [128ti, 128ti+128):
    rows 80j..80j+80 belong to head-half j (0..3 q heads, 4..7 k heads).
    Returns (src_lo, hh, dst_lo, n) DMA copies."""
    out = []
    glo, ghi = 128 * ti, 128 * (ti + 1)
    j0, j1 = glo // HD, (ghi - 1) // HD
    for j in range(j0, j1 + 1):
        s, e = max(glo, HD * j), min(ghi, HD * (j + 1))
        out.append((s - glo, j, s - HD * j, e - s))
    return out


def _body(tc, xT, w_qk, w_v, w_o, outT, dbg=None):
    import concourse.mybir as mybir

    nc = tc.nc
    bf = mybir.dt.bfloat16
    f32 = mybir.dt.float32
    Exp = mybir.ActivationFunctionType.Exp

    with tc.tile_pool(name="weights", bufs=1) as wpool, \
         tc.tile_pool(name="persist", bufs=1) as pers, \
         tc.tile_pool(name="psum", bufs=2, space="PSUM") as ps, \
         tc.tile_pool(name="work", bufs=1) as work:
        # ---- input DMAs on two HWDGE queues: weights on the (idle) ACT
        # queue, x on the SP queue, c2=0 halves first so the upfront
        # projections can start ~6us in ----
        xT_t, wqk_t = [], []
        for i in range(NCT):
            t = wpool.tile([128, S], bf, name=f"xT{i}", tag=f"xT{i}")
            nc.sync.dma_start(out=t[:, 0:1024],
                              in_=xT[128 * i:128 * (i + 1), 0:1024])
            xT_t.append(t)
            t = wpool.tile([128, 2 * GDIM], bf, name=f"wqk{i}", tag=f"wqk{i}")
            nc.scalar.dma_start(out=t, in_=w_qk[128 * i:128 * (i + 1), :])
            wqk_t.append(t)
        for i in range(NCT):
            nc.sync.dma_start(out=xT_t[i][:, 1024:2048],
                              in_=xT[128 * i:128 * (i + 1), 1024:2048])
        wv_t = []
        for i in range(NCT):
            t = wpool.tile([128, GDIM], bf, name=f"wv{i}", tag=f"wv{i}")
            nc.gpsimd.dma_start(out=t, in_=w_v[128 * i:128 * (i + 1), :])
            wv_t.append(t)
        wo_t = []
        for i in range(3):
            t = wpool.tile([128, D], bf, name=f"wo{i}", tag=f"wo{i}")
            nc.gpsimd.dma_start(out=t, in_=w_o[128 * i:128 * (i + 1), :])
            wo_t.append(t)

        # ---- persistent SBUF ----
        qkT = [pers.tile([HD, S], bf, name=f"qkT{i}", tag=f"qkT{i}")
               for i in range(2 * HPC)]
        vaug = [pers.tile([128, VW * NKT], bf, name=f"va{h}", tag=f"va{h}")
                for h in range(HPC)]
        for h in range(HPC):
            nc.vector.memset(vaug[h], 0.0)
            nc.vector.memset(vaug[h][:, VW - 1:VW * NKT:VW], 1.0)
        # packed-96 normalized attention, 3 tiles of [128, S]. Zeroed so the
        # never-written pad rows can't feed NaN into the out-projection
        # (its weights there are zero, but 0*NaN = NaN on the PE).
        attn_p = [pers.tile([128, S], bf, name=f"ap{t}", tag=f"ap{t}")
                  for t in range(3)]
        for t in range(3):
            nc.vector.memset(attn_p[t], 0.0)
        # head 0's q chunks stay in these tiles (rows 0:80) — no repack.
        qk0c = [pers.tile([128, 1024], bf, name=f"qk0c{c}", tag=f"qk0c{c}")
                for c in range(2)]

        def emit_qk_proj(ti, c2, on_act=False):
            # packed q|k projection: output dims [128ti, 128ti+128), one
            # 1024-wide query chunk; copy to SBUF, DMA repack per head.
            # on_act: use the Scalar engine for the copy (idle pre-attention).
            sc = ps.tile([128, 1024], f32, name="qkps", tag="sc")
            for half in range(2):
                cs = slice(1024 * c2 + 512 * half, 1024 * c2 + 512 * (half + 1))
                for k in range(NCT):
                    nc.tensor.matmul(
                        sc[:, 512 * half:512 * (half + 1)],
                        wqk_t[k][:, 128 * ti:128 * (ti + 1)], xT_t[k][:, cs],
                        start=(k == 0), stop=(k == NCT - 1))
            if ti == 0:
                qkp = qk0c[c2]   # head 0 reads rows 0:80 in place
            else:
                qkp = work.tile([128, 1024], bf, name="qkp", tag="qkp", bufs=2)
            if on_act:
                nc.scalar.copy(out=qkp, in_=sc)
            else:
                nc.vector.tensor_copy(out=qkp, in_=sc)
            for slo, hh, dlo, n in _qk_dma_splits(ti):
                if ti == 0 and hh == 0:
                    continue
                nc.sync.dma_start(
                    out=qkT[hh][dlo:dlo + n, 1024 * c2:1024 * (c2 + 1)],
                    in_=qkp[slo:slo + n, :])

        def emit_v_proj(kt):
            # v projection for seq tile kt -> vaug[h] stationaries; copies
            # split ACT/DVE so neither engine paces the PE here.
            sc = ps.tile([128, 1024], f32, name="vps", tag="sc")
            for k in range(NCT):
                nc.tensor.matmul(sc[:, 0:GDIM], xT_t[k][:, 128 * kt:128 * (kt + 1)],
                                 wv_t[k], start=(k == 0), stop=(k == NCT - 1))
            for h in range(HPC):
                eng = nc.scalar if h < 2 else nc.vector
                if eng is nc.scalar:
                    nc.scalar.copy(out=vaug[h][:, VW * kt:VW * kt + HD],
                                   in_=sc[:, HD * h:HD * (h + 1)])
                else:
                    nc.vector.tensor_copy(out=vaug[h][:, VW * kt:VW * kt + HD],
                                          in_=sc[:, HD * h:HD * (h + 1)])

        def emit_out_proj(dt, c2, pair=None):
            # Accumulation order ti=0,1,2: only ti=2 (attn tiles holding head
            # 3) depends on the final chunk's normalize — emitting the ti<2
            # steps of two tiles first hides that latency (see tail callers).
            def chain(po, dt_, tis):
                for ti in tis:
                    nc.tensor.matmul(
                        po[:, :512], wo_t[ti][:, 128 * dt_:128 * (dt_ + 1)],
                        attn_p[ti][:, 1024 * c2:1024 * c2 + 512],
                        start=(ti == 0), stop=(ti == 2))
                    nc.tensor.matmul(
                        po[:, 512:], wo_t[ti][:, 128 * dt_:128 * (dt_ + 1)],
                        attn_p[ti][:, 1024 * c2 + 512:1024 * (c2 + 1)],
                        start=(ti == 0), stop=(ti == 2))

            def finish(po, dt_):
                chain(po, dt_, (2,))
                ob = work.tile([128, 1024], bf, name="ob", tag="ob", bufs=3)
                nc.vector.tensor_copy(out=ob, in_=po)
                nc.sync.dma_start(
                    out=outT[128 * dt_:128 * (dt_ + 1),
                             1024 * c2:1024 * (c2 + 1)],
                    in_=ob)

            po = ps.tile([128, 1024], f32, name="po", tag="sc")
            chain(po, dt, (0, 1))
            if pair is None:
                finish(po, dt)
            else:
                po2 = ps.tile([128, 1024], f32, name="po2", tag="sc")
                chain(po2, pair, (0, 1))
                finish(po, dt)
                finish(po2, pair)

        # upfront: what scores (h0, c2=0, kt<8) need — q0 cols 0:1024
        # (tile 0), k0 cols 0:1024 (tiles 2,3) — plus the whole v
        # projection (PE runs it back-to-back; inside the attention loop
        # it would serialize against the exp stream).
        for ti, c2 in ((0, 0), (2, 0), (3, 0)):
            emit_qk_proj(ti, c2, on_act=True)
        for kt in range(NKT):
            emit_v_proj(kt)

        # insertable work, popped at fixed kt slots of the attention loop.
        def qk(ti, c2):
            return lambda: emit_qk_proj(ti, c2)

        # Insert placement: chunks 0-1 are PE-bound anyway (required
        # projections), so extra work there is free; chunks 2-7 run with
        # ACT ~98% saturated and any inserted PE work would delay the
        # in-order scores stream and stall the exp pipeline - keep them
        # clean except the late out-projection tiles.
        queue = [
            # chunk 0 (h0,c0): k0 cols 1024:2048 (needed from kt=8),
            # q1 rows for (h1,c0)
            [qk(2, 1), qk(3, 1), qk(1, 0)],
            # chunk 1 (h1,c0): q c2=1 tiles (chunk 4), k heads 2,3 (chunk 2)
            [qk(0, 1), qk(1, 1), qk(4, 0), qk(4, 1)],
            [], [], [],          # (h2,c0), (h3,c0), (h0,c1)
            [lambda: emit_out_proj(0, 0), lambda: emit_out_proj(1, 0)],
            [lambda: emit_out_proj(2, 0)],
            [],
        ]

        ci = 0
        for c2 in range(2):
            for h in range(HPC):
                inserts = queue[ci] if ci < len(queue) else []
                ci += 1
                slots = {1: 0, 3: 1, 7: 2, 11: 3, 14: 4}
                at_ = ps.tile([128, 1024], f32, name="at", tag="at")
                pbs = []
                for kt in range(NKT):
                    sc = ps.tile([128, 1024], f32, name="sc", tag="sc")
                    for half in range(2):
                        if h == 0:
                            rhs = qk0c[c2][0:HD, 512 * half:512 * (half + 1)]
                        else:
                            rhs = qkT[h][:, 1024 * c2 + 512 * half:
                                         1024 * c2 + 512 * (half + 1)]
                        nc.tensor.matmul(
                            sc[:, 512 * half:512 * (half + 1)],
                            qkT[HPC + h][:, 128 * kt:128 * (kt + 1)],
                            rhs, start=True, stop=True)
                    pb = work.tile([128, 1024], bf, name="pb", tag=f"pb{kt % 6}",
                                   bufs=1)
                    nc.scalar.activation(out=pb, in_=sc, func=Exp, scale=SM_SCALE)
                    pbs.append(pb)
                    if kt > 0:
                        _emit_pv(nc, at_, vaug[h], pbs[kt - 1], kt - 1)
                    if kt in slots and slots[kt] < len(inserts):
                        fn = inserts[slots[kt]]
                        if fn is not None:
                            fn()
                _emit_pv(nc, at_, vaug[h], pbs[NKT - 1], NKT - 1)
                # normalize: denominators sit at PSUM partition 96. For the
                # last chunk split into 512-halves to halve the tail latency.
                nhalves = 2 if (c2 == 1 and h == HPC - 1) else 1
                w_ = 1024 // nhalves
                for hv in range(nhalves):
                    cs = slice(w_ * hv, w_ * (hv + 1))
                    rdin = work.tile([1, w_], f32, name="rdin", tag="rdin", bufs=2)
                    nc.vector.tensor_copy(out=rdin, in_=at_[96:97, cs])
                    rdr = work.tile([1, w_], f32, name="rdr", tag="rdr", bufs=2)
                    with nc.allow_low_precision(reason="softmax recip, 51 ULP"):
                        nc.vector.reciprocal_approx_fast(out=rdr, in_=rdin)
                    rb = work.tile([HD, w_], f32, name="rb", tag="rb", bufs=2)
                    nc.gpsimd.partition_broadcast(rb, rdr)
                    for t, tlo, alo, n in _head_segments(h):
                        nc.vector.tensor_mul(
                            out=attn_p[t][tlo:tlo + n,
                                          1024 * c2 + w_ * hv:
                                          1024 * c2 + w_ * (hv + 1)],
                            in0=at_[alo:alo + n, cs], in1=rb[alo:alo + n, :])
                if dbg is not None and h == 0 and c2 == 0:
                    atc = work.tile([128, 1024], f32, name="atc", tag="atc")
                    nc.vector.tensor_copy(out=atc, in_=at_)
                    nc.sync.dma_start(out=dbg["at0"], in_=atc)
                    nc.sync.dma_start(out=dbg["rdr0"], in_=rdr)
                    nc.sync.dma_start(out=dbg["rb0"], in_=rb)

        # tail: the last two c2=0 output tiles cover the final chunk's
        # normalize latency, then the c2=1 output projection with paired
        # chains so the head-3-dependent steps land last.
        for dt in (3, 4):
            emit_out_proj(dt, 0)
        emit_out_proj(0, 1, pair=1)
        emit_out_proj(2, 1, pair=3)
        emit_out_proj(4, 1)


def _emit_pv(nc, at_, vaug_h, pb, kt):
    for half in range(2):
        nc.tensor.matmul(
            at_[0:VW, 512 * half:512 * (half + 1)],
            vaug_h[:, VW * kt:VW * (kt + 1)],
            pb[:, 512 * half:512 * (half + 1)],
            start=(kt == 0), stop=(kt == NKT - 1))


def build_nc(loop=1, debug=False):
    import concourse.mybir as mybir
    import concourse.tile as tile
    from concourse import bacc

    bf = mybir.dt.bfloat16
    f32 = mybir.dt.float32
    nc = bacc.Bacc("TRN2", target_bir_lowering=False, debug=False,
                   num_devices=NCORES)
    xT = nc.dram_tensor("xT", [D, S], bf, kind="ExternalInput").ap()
    w_qk = nc.dram_tensor("w_qk", [D, 2 * GDIM], bf, kind="ExternalInput").ap()
    w_v = nc.dram_tensor("w_v", [D, GDIM], bf, kind="ExternalInput").ap()
    w_o = nc.dram_tensor("w_o", [3 * 128, D], bf, kind="ExternalInput").ap()
    outT = nc.dram_tensor("outT", [D, S], bf, kind="ExternalOutput").ap()
    dbg = None
    if debug:
        dbg = {
            "at0": nc.dram_tensor("at0", [128, 1024], f32, kind="ExternalOutput").ap(),
            "rdr0": nc.dram_tensor("rdr0", [1, 1024], f32, kind="ExternalOutput").ap(),
            "rb0": nc.dram_tensor("rb0", [HD, 1024], f32, kind="ExternalOutput").ap(),
        }
    with tile.TileContext(nc) as tc:
        if loop == 1:
            _body(tc, xT, w_qk, w_v, w_o, outT, dbg)
        else:
            with tc.For_i(0, loop, 1):
                _body(tc, xT, w_qk, w_v, w_o, outT)
    nc.compile()
    return nc


def make_in_maps(inputs):
    """Host-side shard + layout prep. inputs: full-size fp32 arrays."""
    f = {k: np.asarray(v, dtype=np.float64) for k, v in inputs.items()}
    w_eff = {}
    for nm in ("q", "k", "v", "o"):
        w_eff[nm] = (f[f"w{nm}"] + f[f"{nm}_up"] @ f[f"{nm}_down"])
    bfd = ml_dtypes.bfloat16
    x = f["hidden_states"]  # [B, S, D]
    in_maps = []
    for c in range(NCORES):
        b, g = divmod(c, 2)
        rows = slice(GDIM * g, GDIM * (g + 1))
        xT_ = np.ascontiguousarray(x[b].T).astype(bfd)
        wq = w_eff["q"][rows, :].T  # [640, 320]
        wk = w_eff["k"][rows, :].T
        w_qk = np.ascontiguousarray(np.concatenate([wq, wk], axis=1)).astype(bfd)
        w_v = np.ascontiguousarray(w_eff["v"][rows, :].T).astype(bfd)
        # packed-96 w_o: rows 96h..96h+80 = head h's 80 contraction rows,
        # pad rows zero so they contribute nothing.
        wo_pack = np.zeros((3 * 128, D), np.float64)
        for h in range(HPC):
            wo_pack[PAD * h:PAD * h + HD, :] = \
                w_eff["o"][:, GDIM * g + HD * h:GDIM * g + HD * (h + 1)].T
        in_maps.append({"xT": xT_, "w_qk": w_qk, "w_v": w_v,
                        "w_o": wo_pack.astype(bfd)})
    return in_maps


def assemble_out(results, bo):
    out = np.empty((B, S, D), np.float32)
    for b in range(B):
        pt = (results[2 * b]["outT"].astype(np.float32)
              + results[2 * b + 1]["outT"].astype(np.float32))  # [640, 2048]
        out[b] = pt.T + bo[None, :].astype(np.float32)
    return out


def kernel(**inputs):
    from concourse.bass_utils import run_bass_kernel_spmd

    if "nc" not in _cache:
        _cache["nc"] = build_nc()
    nc = _cache["nc"]
    in_maps = make_in_maps(inputs)
    res = run_bass_kernel_spmd(nc, in_maps, list(range(NCORES)))
    return assemble_out(res.results, np.asarray(inputs["bo"], np.float32))


# revision 40
# speedup vs baseline: 1.0814x; 1.0728x over previous
"""LoRA cross-attention (self-attn) processor on 8 TRN2 NeuronCores.

Problem: B=4, S=2048, D=640, H=8 heads (hd=80), LoRA rank 4.
  q/k/v = x @ (W + up@down).T ; per-head attention; out = attn @ (Wo + o_up@o_down).T + bo

Sharding: batch*head parallel. Core c -> batch b=c//2, head-group g=c%2
(4 heads). Host folds the rank-4 LoRA updates into the weights (exact
algebra) and sums the two partial outputs per batch + bias at the end.

Schedule (all matmul operands bf16, PSUM fp32). The kernel is co-limited
by the PE (~370k matmul columns ~ 154us at 2.4GHz) and the Scalar
engine (128 exp instructions over 16.8M scores ~ 142us), under a chip
power throttle (~0.85 avg util). Design points:
  - q/k projection runs packed (M=128 per matmul, 5 output tiles of the
    640 q|k dims), then SBUF->SBUF DMA repacks rows into per-head-half
    [80, S] tiles (DMA has no partition-alignment restriction; engine
    APs would need 32-aligned bases, impossible at 80-row strides).
    Head 0's q stays in the packed tile and is read in place.
  - q-chunks of 1024: scores tiles [128k, 1024q] in 2-bank PSUM tiles,
    one ACT exp per tile (halves ACT per-instruction overhead vs 512).
    ACT runs exp only; copies go to DVE (or ACT when it is idle).
  - v stationary [128, 97] with a ones column at col 96: PV emits the
    softmax denominators at PSUM partition 96 (32-aligned, directly
    readable). Normalize = DVE copy + reciprocal_approx_fast [1,1024]
    + GPSIMD partition_broadcast + DVE multiplies; fp32 throughout.
  - attention output packed at 96-partition stride across 3 SBUF tiles
    (every head-segment boundary lands 32-aligned); output projection
    contracts 3x128 packed rows against host-zero-padded W_o (30720
    columns instead of 40960).
  - inputs stream on three DMA queues (SP: x halves c0-first, ACT
    HWDGE: w_qk, GPSIMD SWDGE: w_v/w_o) so the first projection chain
    starts ~2us in; upfront work is only head-0's needs + the v
    projection; all other projections, repacks and the first
    out-projection pass are fed into attention-loop insert slots so the
    PE never idles long enough for the HAM clock gate to drop.
  - out-projection accumulation is ordered ti=0,1,2 and paired so the
    steps depending on the last chunk's normalize land last.
  - output written bf16 (host accumulates the two partials in fp32).

PSUM = exactly 8 banks: tags sc(x2) + at(x2), all [128,1024] fp32.
Measured (NTFF, core 0): ~250us total / ~230us active vs 359us baseline.
"""
import numpy as np
import ml_dtypes

B, S, D, H, HD, R = 4, 2048, 640, 8, 80, 4
HPC = H // 2          # heads per core
GDIM = HPC * HD       # 320 head-dims per core
PAD = 96              # packed head stride (all segment bases 32-aligned)
NCORES = 8
NKT = S // 128        # 16 key tiles
NCT = D // 128        # 5 contraction tiles
VW = 97               # v stationary width: 80 dims + pad + ones col at 96
SM_SCALE = 1.0 / float(np.sqrt(HD))

_cache = {}


def _seg_legal(base, n):
    # Engine partition-range rule: n<=32 from any 32-aligned base,
    # n<=64 from {0,64}, larger only from 0.
    if n <= 32:
        return base % 32 == 0
    if n <= 64:
        return base in (0, 64)
    return base == 0


def _split_legal(pairs):
    """Split (off_a, off_b, n) ranges so every piece is engine-legal in
    both coordinates (all offsets here are multiples of 32, so one
    32-grid serves both)."""
    pieces = []
    for a, b, n in pairs:
        o = 0
        while o < n:
            m = n - o
            while m > 1 and not (_seg_legal(a + o, m) and _seg_legal(b + o, m)):
                m = 32 * ((m - 1) // 32) if m > 32 else m - 1
            pieces.append((a + o, b + o, m))
            o += m
    return pieces


def _head_segments(h):
    """Packed-96 attn layout: head h occupies packed rows [96h, 96h+80)
    across three 128-partition tiles -> (tile, tile_lo, at_lo, n)."""
    segs = []
    lo, hi = PAD * h, PAD * h + HD
    for t in range(3):
        s, e = max(lo, 128 * t), min(hi, 128 * (t + 1))
        if s < e:
            for tl, al, n in _split_legal([(s - 128 * t, s - lo, e - s)]):
                segs.append((t, tl, al, n))
    return segs


def _qk_dma_splits(ti):
    """Packed q|k projection tile ti covers global dims [128ti, 128ti+128):
    rows 80j..80j+80 belong to head-half j (0..3 q heads, 4..7 k heads).
    Returns (src_lo, hh, dst_lo, n) DMA copies."""
    out = []
    glo, ghi = 128 * ti, 128 * (ti + 1)
    j0, j1 = glo // HD, (ghi - 1) // HD
    for j in range(j0, j1 + 1):
        s, e = max(glo, HD * j), min(ghi, HD * (j + 1))
        out.append((s - glo, j, s - HD * j, e - s))
    return out


def _body(tc, xT, w_qk, w_v, w_o, outT, dbg=None):
    import concourse.mybir as mybir

    nc = tc.nc
    bf = mybir.dt.bfloat16
    f32 = mybir.dt.float32
    Exp = mybir.ActivationFunctionType.Exp

    with tc.tile_pool(name="weights", bufs=1) as wpool, \
         tc.tile_pool(name="persist", bufs=1) as pers, \
         tc.tile_pool(name="psum", bufs=2, space="PSUM") as ps, \
         tc.tile_pool(name="work", bufs=1) as work:
        # ---- input DMAs on two HWDGE queues: weights on the (idle) ACT
        # queue, x on the SP queue, c2=0 halves first so the upfront
        # projections can start ~6us in ----
        xT_t, wqk_t = [], []
        for i in range(NCT):
            t = wpool.tile([128, S], bf, name=f"xT{i}", tag=f"xT{i}")
            nc.sync.dma_start(out=t[:, 0:1024],
                              in_=xT[128 * i:128 * (i + 1), 0:1024])
            xT_t.append(t)
            t = wpool.tile([128, 2 * GDIM], bf, name=f"wqk{i}", tag=f"wqk{i}")
            nc.scalar.dma_start(out=t, in_=w_qk[128 * i:128 * (i + 1), :])
            wqk_t.append(t)
        for i in range(NCT):
            nc.sync.dma_start(out=xT_t[i][:, 1024:2048],
                              in_=xT[128 * i:128 * (i + 1), 1024:2048])
        wv_t = []
        for i in range(NCT):
            t = wpool.tile([128, GDIM], bf, name=f"wv{i}", tag=f"wv{i}")
            nc.gpsimd.dma_start(out=t, in_=w_v[128 * i:128 * (i + 1), :])
            wv_t.append(t)
        wo_t = []
        for i in range(3):
            t = wpool.tile([128, D], bf, name=f"wo{i}", tag=f"wo{i}")
            nc.gpsimd.dma_start(out=t, in_=w_o[128 * i:128 * (i + 1), :])
            wo_t.append(t)

        # ---- persistent SBUF ----
        qkT = [pers.tile([HD, S], bf, name=f"qkT{i}", tag=f"qkT{i}")
               for i in range(2 * HPC)]
        vaug = [pers.tile([128, VW * NKT], bf, name=f"va{h}", tag=f"va{h}")
                for h in range(HPC)]
        for h in range(HPC):
            nc.vector.memset(vaug[h], 0.0)
            nc.vector.memset(vaug[h][:, VW - 1:VW * NKT:VW], 1.0)
        # packed-96 normalized attention, 3 tiles of [128, S]. Zeroed so the
        # never-written pad rows can't feed NaN into the out-projection
        # (its weights there are zero, but 0*NaN = NaN on the PE).
        attn_p = [pers.tile([128, S], bf, name=f"ap{t}", tag=f"ap{t}")
                  for t in range(3)]
        for t in range(3):
            nc.vector.memset(attn_p[t], 0.0)
        # head 0's q chunks stay in these tiles (rows 0:80) — no repack.
        qk0c = [pers.tile([128, 1024], bf, name=f"qk0c{c}", tag=f"qk0c{c}")
                for c in range(2)]

        def emit_qk_proj(ti, c2, on_act=False):
            # packed q|k projection: output dims [128ti, 128ti+128), one
            # 1024-wide query chunk; copy to SBUF, DMA repack per head.
            # on_act: use the Scalar engine for the copy (idle pre-attention).
            sc = ps.tile([128, 1024], f32, name="qkps", tag="sc")
            for half in range(2):
                cs = slice(1024 * c2 + 512 * half, 1024 * c2 + 512 * (half + 1))
                for k in range(NCT):
                    nc.tensor.matmul(
                        sc[:, 512 * half:512 * (half + 1)],
                        wqk_t[k][:, 128 * ti:128 * (ti + 1)], xT_t[k][:, cs],
                        start=(k == 0), stop=(k == NCT - 1))
            if ti == 0:
                qkp = qk0c[c2]   # head 0 reads rows 0:80 in place
            else:
                qkp = work.tile([128, 1024], bf, name="qkp", tag="qkp", bufs=2)
            if on_act:
                nc.scalar.copy(out=qkp, in_=sc)
            else:
                nc.vector.tensor_copy(out=qkp, in_=sc)
            for slo, hh, dlo, n in _qk_dma_splits(ti):
                if ti == 0 and hh == 0:
                    continue
                nc.sync.dma_start(
                    out=qkT[hh][dlo:dlo + n, 1024 * c2:1024 * (c2 + 1)],
                    in_=qkp[slo:slo + n, :])

        def emit_v_proj(kt):
            # v projection for seq tile kt -> vaug[h] stationaries; copies
            # split ACT/DVE so neither engine paces the PE here.
            sc = ps.tile([128, 1024], f32, name="vps", tag="sc")
            for k in range(NCT):
                nc.tensor.matmul(sc[:, 0:GDIM], xT_t[k][:, 128 * kt:128 * (kt + 1)],
                                 wv_t[k], start=(k == 0), stop=(k == NCT - 1))
            for h in range(HPC):
                eng = nc.scalar if h < 2 else nc.vector
                if eng is nc.scalar:
                    nc.scalar.copy(out=vaug[h][:, VW * kt:VW * kt + HD],
                                   in_=sc[:, HD * h:HD * (h + 1)])
                else:
                    nc.vector.tensor_copy(out=vaug[h][:, VW * kt:VW * kt + HD],
                                          in_=sc[:, HD * h:HD * (h + 1)])

        def emit_out_proj(dt, c2, pair=None):
            # Accumulation order ti=0,1,2: only ti=2 (attn tiles holding head
            # 3) depends on the final chunk's normalize — emitting the ti<2
            # steps of two tiles first hides that latency (see tail callers).
            def chain(po, dt_, tis):
                for ti in tis:
                    nc.tensor.matmul(
                        po[:, :512], wo_t[ti][:, 128 * dt_:128 * (dt_ + 1)],
                        attn_p[ti][:, 1024 * c2:1024 * c2 + 512],
                        start=(ti == 0), stop=(ti == 2))
                    nc.tensor.matmul(
                        po[:, 512:], wo_t[ti][:, 128 * dt_:128 * (dt_ + 1)],
                        attn_p[ti][:, 1024 * c2 + 512:1024 * (c2 + 1)],
                        start=(ti == 0), stop=(ti == 2))

            def finish(po, dt_):
                chain(po, dt_, (2,))
                ob = work.tile([128, 1024], bf, name="ob", tag="ob", bufs=3)
                nc.vector.tensor_copy(out=ob, in_=po)
                nc.sync.dma_start(
                    out=outT[128 * dt_:128 * (dt_ + 1),
                             1024 * c2:1024 * (c2 + 1)],
                    in_=ob)

            po = ps.tile([128, 1024], f32, name="po", tag="sc")
            chain(po, dt, (0, 1))
            if pair is None:
                finish(po, dt)
            else:
                po2 = ps.tile([128, 1024], f32, name="po2", tag="sc")
                chain(po2, pair, (0, 1))
                finish(po, dt)
                finish(po2, pair)

        # upfront: what scores (h0, c2=0, kt<8) need — q0 cols 0:1024
        # (tile 0), k0 cols 0:1024 (tiles 2,3) — plus the whole v
        # projection (PE runs it back-to-back; inside the attention loop
        # it would serialize against the exp stream).
        for ti, c2 in ((0, 0), (2, 0), (3, 0)):
            emit_qk_proj(ti, c2, on_act=True)
        for kt in range(NKT):
            emit_v_proj(kt)

        # insertable work, popped at fixed kt slots of the attention loop.
        def qk(ti, c2):
            return lambda: emit_qk_proj(ti, c2)

        # Insert placement: chunks 0-1 are PE-bound anyway (required
        # projections), so extra work there is free; chunks 2-7 run with
        # ACT ~98% saturated and any inserted PE work would delay the
        # in-order scores stream and stall the exp pipeline - keep them
        # clean except the late out-projection tiles.
        queue = [
            # chunk 0 (h0,c0): k0 cols 1024:2048 (needed from kt=8),
            # q1 rows for (h1,c0)
            [qk(2, 1), qk(3, 1), qk(1, 0)],
            # chunk 1 (h1,c0): q c2=1 tiles (chunk 4), k heads 2,3 (chunk 2)
            [qk(0, 1), qk(1, 1), qk(4, 0), qk(4, 1)],
            [], [], [],          # (h2,c0), (h3,c0), (h0,c1)
            [lambda: emit_out_proj(0, 0), lambda: emit_out_proj(1, 0)],
            [lambda: emit_out_proj(2, 0)],
            [],
        ]

        def emit_norm(at_, h, c2, last):
            # normalize: denominators sit at PSUM partition 96. For the
            # last chunk split into 512-halves to halve the tail latency.
            nhalves = 2 if last else 1
            w_ = 1024 // nhalves
            for hv in range(nhalves):
                cs = slice(w_ * hv, w_ * (hv + 1))
                rdin = work.tile([1, w_], f32, name="rdin", tag="rdin", bufs=2)
                nc.vector.tensor_copy(out=rdin, in_=at_[96:97, cs])
                rdr = work.tile([1, w_], f32, name="rdr", tag="rdr", bufs=2)
                with nc.allow_low_precision(reason="softmax recip, 51 ULP"):
                    nc.vector.reciprocal_approx_fast(out=rdr, in_=rdin)
                rb = work.tile([HD, w_], f32, name="rb", tag="rb", bufs=2)
                nc.gpsimd.partition_broadcast(rb, rdr)
                for t, tlo, alo, n in _head_segments(h):
                    nc.vector.tensor_mul(
                        out=attn_p[t][tlo:tlo + n,
                                      1024 * c2 + w_ * hv:
                                      1024 * c2 + w_ * (hv + 1)],
                        in0=at_[alo:alo + n, cs], in1=rb[alo:alo + n, :])

        # Flat software pipeline over all (chunk, kt) steps with the PV
        # matmuls lagging two steps behind the score/exp stream. The lag
        # carries across chunk boundaries, so sc/exp of the next chunk are
        # emitted BEFORE the last PV pair of the previous chunk — without
        # this, pv(old,15) waits ~1.1us on exp(old,15) while ACT starves.
        pvq = []

        def drain_pv(keep):
            while len(pvq) > keep:
                a_, vh_, pb_, k_, norm_fn = pvq.pop(0)
                _emit_pv(nc, a_, vh_, pb_, k_)
                if norm_fn is not None:
                    norm_fn()

        ci = 0
        for c2 in range(2):
            for h in range(HPC):
                inserts = queue[ci] if ci < len(queue) else []
                last_chunk = ci == 2 * HPC - 1
                ci += 1
                slots = {1: 0, 3: 1, 7: 2, 11: 3, 14: 4}
                at_ = ps.tile([128, 1024], f32, name="at", tag="at")
                for kt in range(NKT):
                    sc = ps.tile([128, 1024], f32, name="sc", tag="sc")
                    for half in range(2):
                        if h == 0:
                            rhs = qk0c[c2][0:HD, 512 * half:512 * (half + 1)]
                        else:
                            rhs = qkT[h][:, 1024 * c2 + 512 * half:
                                         1024 * c2 + 512 * (half + 1)]
                        nc.tensor.matmul(
                            sc[:, 512 * half:512 * (half + 1)],
                            qkT[HPC + h][:, 128 * kt:128 * (kt + 1)],
                            rhs, start=True, stop=True)
                    pb = work.tile([128, 1024], bf, name="pb", tag=f"pb{kt % 6}",
                                   bufs=1)
                    nc.scalar.activation(out=pb, in_=sc, func=Exp, scale=SM_SCALE)
                    norm_fn = None
                    if kt == NKT - 1:
                        norm_fn = (lambda a=at_, hh=h, cc=c2, ll=last_chunk:
                                   emit_norm(a, hh, cc, ll))
                    pvq.append((at_, vaug[h], pb, kt, norm_fn))
                    drain_pv(2)
                    if kt in slots and slots[kt] < len(inserts):
                        fn = inserts[slots[kt]]
                        if fn is not None:
                            fn()
        drain_pv(0)

        # tail: the last two c2=0 output tiles cover the final chunk's
        # normalize latency, then the c2=1 output projection with paired
        # chains so the head-3-dependent steps land last.
        for dt in (3, 4):
            emit_out_proj(dt, 0)
        emit_out_proj(0, 1, pair=1)
        emit_out_proj(2, 1, pair=3)
        emit_out_proj(4, 1)


def _emit_pv(nc, at_, vaug_h, pb, kt):
    for half in range(2):
        nc.tensor.matmul(
            at_[0:VW, 512 * half:512 * (half + 1)],
            vaug_h[:, VW * kt:VW * (kt + 1)],
            pb[:, 512 * half:512 * (half + 1)],
            start=(kt == 0), stop=(kt == NKT - 1))


def build_nc(loop=1, debug=False):
    import concourse.mybir as mybir
    import concourse.tile as tile
    from concourse import bacc

    bf = mybir.dt.bfloat16
    f32 = mybir.dt.float32
    nc = bacc.Bacc("TRN2", target_bir_lowering=False, debug=False,
                   num_devices=NCORES)
    xT = nc.dram_tensor("xT", [D, S], bf, kind="ExternalInput").ap()
    w_qk = nc.dram_tensor("w_qk", [D, 2 * GDIM], bf, kind="ExternalInput").ap()
    w_v = nc.dram_tensor("w_v", [D, GDIM], bf, kind="ExternalInput").ap()
    w_o = nc.dram_tensor("w_o", [3 * 128, D], bf, kind="ExternalInput").ap()
    outT = nc.dram_tensor("outT", [D, S], bf, kind="ExternalOutput").ap()
    dbg = None
    if debug:
        dbg = {
            "at0": nc.dram_tensor("at0", [128, 1024], f32, kind="ExternalOutput").ap(),
            "rdr0": nc.dram_tensor("rdr0", [1, 1024], f32, kind="ExternalOutput").ap(),
            "rb0": nc.dram_tensor("rb0", [HD, 1024], f32, kind="ExternalOutput").ap(),
        }
    with tile.TileContext(nc) as tc:
        if loop == 1:
            _body(tc, xT, w_qk, w_v, w_o, outT, dbg)
        else:
            with tc.For_i(0, loop, 1):
                _body(tc, xT, w_qk, w_v, w_o, outT)
    nc.compile()
    return nc


def make_in_maps(inputs):
    """Host-side shard + layout prep. inputs: full-size fp32 arrays."""
    f = {k: np.asarray(v, dtype=np.float64) for k, v in inputs.items()}
    w_eff = {}
    for nm in ("q", "k", "v", "o"):
        w_eff[nm] = (f[f"w{nm}"] + f[f"{nm}_up"] @ f[f"{nm}_down"])
    bfd = ml_dtypes.bfloat16
    x = f["hidden_states"]  # [B, S, D]
    in_maps = []
    for c in range(NCORES):
        b, g = divmod(c, 2)
        rows = slice(GDIM * g, GDIM * (g + 1))
        xT_ = np.ascontiguousarray(x[b].T).astype(bfd)
        wq = w_eff["q"][rows, :].T  # [640, 320]
        wk = w_eff["k"][rows, :].T
        w_qk = np.ascontiguousarray(np.concatenate([wq, wk], axis=1)).astype(bfd)
        w_v = np.ascontiguousarray(w_eff["v"][rows, :].T).astype(bfd)
        # packed-96 w_o: rows 96h..96h+80 = head h's 80 contraction rows,
        # pad rows zero so they contribute nothing.
        wo_pack = np.zeros((3 * 128, D), np.float64)
        for h in range(HPC):
            wo_pack[PAD * h:PAD * h + HD, :] = \
                w_eff["o"][:, GDIM * g + HD * h:GDIM * g + HD * (h + 1)].T
        in_maps.append({"xT": xT_, "w_qk": w_qk, "w_v": w_v,
                        "w_o": wo_pack.astype(bfd)})
    return in_maps


def assemble_out(results, bo):
    out = np.empty((B, S, D), np.float32)
    for b in range(B):
        pt = (results[2 * b]["outT"].astype(np.float32)
              + results[2 * b + 1]["outT"].astype(np.float32))  # [640, 2048]
        out[b] = pt.T + bo[None, :].astype(np.float32)
    return out


def kernel(**inputs):
    from concourse.bass_utils import run_bass_kernel_spmd

    if "nc" not in _cache:
        _cache["nc"] = build_nc()
    nc = _cache["nc"]
    in_maps = make_in_maps(inputs)
    res = run_bass_kernel_spmd(nc, in_maps, list(range(NCORES)))
    return assemble_out(res.results, np.asarray(inputs["bo"], np.float32))
